# revision 25
# baseline (speedup 1.0000x reference)
"""MinGRU layer on 8 Trainium2 NeuronCores (batch-parallel).

Math (per batch b, reference semantics, all fp32):
    g = sigmoid(x @ Wg.T + bg)
    a = sigmoid(x @ Wd.T + bd)
    v = x @ Wv.T + bv
    h_t = a_t * h_{t-1} + (1 - a_t) * v_t     (causal scan over S)
    out = h * g

Design (measured-bottleneck-driven; PE matmul work is ~332us of the ~354us
total, i.e. the kernel runs at ~95% of the bf16 tensor-engine roofline;
steady-state pass gap is exactly 512 PE cycles, zero per-instruction
overhead):

Measured dead ends (don't retry):
  - fp8 DoubleRow runs 512-col passes at the SAME 216ns as bf16 (2x FLOPs
    via K=256/pass, matching the 157TF/s spec; the CoreSim cost model's
    0.5 cycles/row = 4x is wrong on hardware). Precision needs both
    operands split hi+lo (12 passes/proj vs bf16's 8) => 1.5x SLOWER.
    Plain fp8 e4m3 rel-err is 5.3e-2 > 2e-2 tolerance; single-operand
    splits 3-4e-2. fp8 is strictly dominated here.
  - 1024-col matmuls (2 psum banks) are ISA-illegal (walrus NCC_IXCG864).
  - Starting the PE earlier on finer first-x slices loses ~2us: the
    e-tile-0 phase is x-supply-bound (~390GB/s effective, stream done
    ~40us), so an earlier start converts free front idle into mid-stream
    stall events.
  - ~30 periodic +216ns PE stalls at a fixed ~10.8us cadence (~7us) are
    system-level (present in pure-PE microbenches; unfixable).
  - Whole runs execute at a per-run DVFS state (216/235/259 ns per pass =
    2.37/2.18/1.98 GHz), flat for the entire run: measured exec varies
    354k..421k ns for identical code. Compare runs by median pass gap.
  - B=8 batches -> one batch element per NeuronCore (embarrassingly parallel).
  - Host side: x[b] -> X.T [D, S] bf16; weights packed per e-tile into
    [P, 3*KT*P] rows (one contiguous 6KB DMA row per partition); bias packed
    to [P, 4*ET]; all to minimize DMA instruction count (each dma_start costs
    ~0.7us of serialized dispatch on the sync queue — the startup gate).
  - On-chip per core:
      * matmuls produce projections directly in [e(part), s(free)] layout:
        out[e, s] = sum_d W.T[d, e] * X.T[d, s]  via PE (stationary = W.T tile)
      * ACT evicts PSUM with fused bias+sigmoid (gate/decay); DVE evicts the
        value projection as u' = (a-1)*v straight out of PSUM
      * DVE runs the recurrence via the hardware TensorTensorScanArith op:
        state = a*state - u' = a*state + (1-a)*v  (bv folded via init -bv)
      * out = (h'+bv)*g -> bf16, DMA back to DRAM in [e, s]; host transposes
        and upcasts (bf16 halves the final HBM write drain).
  - Schedule: ~110 junk warm-up matmuls burn the TRN2 PE p-state ramp
    (0.65->2.4GHz over ~3us busy) while the first x slices stream in; the
    first e-tile interleaves decay+gate s-major so PE consumption matches
    the x arrival rate; middle e-tiles run decay/value/gate k-major with
    full-tile scans overlapped one group behind; the last e-tile runs
    decay, value s-major (u-evicts + two chained scan halves trail the
    banks), then gate s-major with the out-multiply + per-bank store
    interleaved so earlier chunks drain while later banks stream; the
    final gate bank is two 256-col accumulation groups in one psum bank,
    leaving only ACT(256)+mult(256)+one small store after the last
    matmul (exec ends ~6.3-6.8us after the last matmul; ~4.4us of that
    is the framework sem-teardown epilogue).
  - g/om work tiles are parity-double-buffered to break WAR stalls between
    consecutive e-tiles; weight slabs rotate 3 tags deep, loaded ~2 e-tiles
    ahead.
"""

import os

import numpy as np
import ml_dtypes

B, S, D = 8, 4096, 1024
P = 128
KT = D // P          # 8 contraction tiles
ET = D // P          # 8 output-channel tiles
SCH = 512            # psum free-dim chunk (one bank, fp32)
NS = S // SCH        # 8 s-chunks
XC = 1024            # x DMA column-chunk
NXC = S // XC

_BF16 = ml_dtypes.bfloat16
_E3M4 = ml_dtypes.float8_e3m4

_nc_cache = {}


def _build_nc(bv_zero=True):
    """Build + compile the single-core Bass program (shared by all 8 cores)."""
    key = ("nc", bv_zero)
    if key in _nc_cache:
        return _nc_cache[key]

    from contextlib import ExitStack

    import concourse.bacc as bacc
    import concourse.mybir as mybir
    from concourse import tile

    dt = mybir.dt
    AF = mybir.ActivationFunctionType
    OP = mybir.AluOpType

    nc = bacc.Bacc("TRN2", target_bir_lowering=False, debug=False, num_devices=8)

    # x ships as fp8 E3M4 (5 significant bits), pre-scaled by 2 on the host
    # with the weights pre-divided by 2 (exact exponent shifts) so the
    # products are unchanged. E3M4 is a full-rate matmul moving operand
    # (1.0 cycles/row, same 216ns/pass) and HALVES the x stream (8.4->4.2
    # MB), which gates the startup. Simulated end-to-end error (simulator
    # matches HW to 7 digits on the bf16 config): 1.74e-2 < 2e-2 tol.
    xt = nc.dram_tensor("xt", [D, S], dt.float8e3, kind="ExternalInput").ap()
    # wt[et, p, j*KT*P + k*P + e] = W_j.T[k*P+p, et*P+e]: one contiguous 6KB
    # row per partition per e-tile -> a single cheap DMA per e-tile.
    # j: 0=decay(Wd), 1=value(Wv), 2=gate(Wg)
    wt = nc.dram_tensor("wt", [ET, P, 3 * KT * P], dt.bfloat16, kind="ExternalInput").ap()
    # bias pre-packed host-side to [P, 4*ET]: col j*ET+e holds bias_j[e*P+p];
    # j: 0=bd, 1=bv, 2=bg, 3=-bv  (bv folded into the scan: h = h'+bv,
    # h' scans (a-1)*(xWv) with initial -bv, out = (h'+bv)*g)
    bias = nc.dram_tensor("bias", [P, 4 * ET], dt.float32, kind="ExternalInput").ap()
    # bf16 output halves the HBM write drain; the host upcasts. Rounding adds
    # <= 0.2% of |out| — far inside the tolerance.
    out = nc.dram_tensor("out", [D, S], dt.bfloat16, kind="ExternalOutput").ap()

    with tile.TileContext(nc) as tc, ExitStack() as ctx:
        xp = ctx.enter_context(tc.tile_pool(name="xp", bufs=1))
        wp = ctx.enter_context(tc.tile_pool(name="wp", bufs=1))
        bp = ctx.enter_context(tc.tile_pool(name="bp", bufs=1))
        work = ctx.enter_context(tc.tile_pool(name="work", bufs=1))
        psum = ctx.enter_context(tc.tile_pool(name="psum", bufs=1, space="PSUM"))

        # Weight slabs rotate through 3 e-tiles' worth of tags; one DMA per
        # e-tile loads all three projections (3 x 2KB rows per partition).
        wtiles = {}

        def _load_w(et, j=None):
            """Load e-tile et's weights; j=None loads all three projections
            in one DMA, j=int loads only that projection's slab (used for
            e-tile 0 so the first matmuls wait on a 256KB transfer, not
            the full 768KB)."""
            if et in wtiles:
                t = wtiles[et]
            else:
                t = wp.tile(
                    [P, 3 * KT * P], dt.bfloat16, tag=f"wt{et % 3}", name=f"w{et}"
                )
                wtiles[et] = t
            if j is None:
                nc.sync.dma_start(t[:], wt[et])
            else:
                sl = slice(j * KT * P, (j + 1) * KT * P)
                nc.sync.dma_start(t[:, sl], wt[et][:, sl])

        def wop(et, j, k):
            return wtiles[et][:, j * KT * P + k * P:(j * KT * P + k * P) + P]

        # x: column-slice tiles, each holding a column range for a k-tile
        # range (one DMA instruction covers many matmul operands; per-DMA
        # dispatch on the sync queue is ~0.6us, the startup gate). The first
        # 512 columns are split into two k-halves so the very first matmuls
        # have a small (512KB) transfer to wait on. (Starting the PE earlier
        # on finer slices was measured to LOSE ~2us: the e-tile-0 phase is
        # x-supply-bound, so an earlier start just converts free front idle
        # into mid-stream stalls with extra restart overhead.)
        XSL = [(0, 4, 0, 512), (4, 8, 0, 512), (0, 4, 512, 1024),
               (4, 8, 512, 1024), (0, 4, 1024, 2048), (4, 8, 1024, 2048),
               (0, 4, 2048, 3072), (4, 8, 2048, 3072), (0, 4, 3072, 4096),
               (4, 8, 3072, 4096)]
        xtile = [None] * len(XSL)
        xt3 = xt.rearrange("(k p) s -> p k s", p=P)

        def _load_x(i):
            k0, k1, c0, c1 = XSL[i]
            t = xp.tile(
                [P, (k1 - k0) * (c1 - c0)], dt.float8e3, tag=f"x{i}", name=f"x{i}"
            )
            nc.sync.dma_start(
                t[:].rearrange("p (k s) -> p k s", k=k1 - k0),
                xt3[:, k0:k1, c0:c1],
            )
            xtile[i] = t

        def xop(k, s):
            c = s * SCH
            for i, (k0, k1, c0, c1) in enumerate(XSL):
                if k0 <= k < k1 and c0 <= c < c1:
                    o = (k - k0) * (c1 - c0) + (c - c0)
                    return xtile[i][:, o:o + SCH]
            raise AssertionError

        # PE warm-up: TRN2 ramps the PE clock 0.65 -> 2.4 GHz over ~3us of
        # busy time. Burn the ramp on junk matmuls over a memset tile while
        # the input DMAs are still in flight (~5us; ends about when the
        # first x slices land).
        warm = bp.tile([P, 64], dt.bfloat16, name="warm")
        nc.gpsimd.memset(warm[:], 0.0)
        wps = psum.tile([64, 64], dt.float32, tag="p0", name="wps")
        for r in range(110):
            nc.tensor.matmul(wps[:], warm[:], warm[:], start=True, stop=True)

        # DMA issue order ~ first-use order. (Keep the instruction count
        # minimal here: each dma_start costs ~0.7us of serialized dispatch
        # on the sync queue, which is the binding constraint at startup.
        # Splitting w0 into per-projection slabs to start the PE earlier
        # was measured to LOSE ~2-6us across three head-to-heads: the DMA
        # rings have a ~3-4us slow-start, so the first matmul is
        # data-gated near ~14us regardless, and the reordered stream
        # creates extra pacing stalls.)
        _load_w(0)
        _load_x(0)
        _load_x(1)
        _load_x(2)
        _load_x(3)
        _load_x(4)
        _load_x(5)
        btile = bp.tile([P, 4 * ET], dt.float32)
        nc.sync.dma_start(btile[:], bias)
        # Dummy sigmoid hoists the ACT table load to kernel start.
        scratch = bp.tile([P, 1], dt.float32, name="scratch")
        nc.scalar.activation(
            scratch[:], btile[:, 0:1], AF.Sigmoid, bias=btile[:, 1:2]
        )
        for i in range(6, len(XSL)):
            _load_x(i)
        _load_w(1)
        _load_w(2)

        # Work tiles: g and om are double-buffered (parity) to break WAR
        # stalls against the previous e-tile's consumers.
        a = work.tile([P, S], dt.float32, tag="a", name="a")
        u = work.tile([P, S], dt.float32, tag="u", name="u")
        h = work.tile([P, S], dt.float32, tag="h", name="h")
        gbuf = [
            work.tile([P, S], dt.float32, tag="g0", name="g0"),
            work.tile([P, S], dt.float32, tag="g1", name="g1"),
        ]
        ombuf = [
            work.tile([P, S], dt.bfloat16, tag="om0", name="om0"),
            work.tile([P, S], dt.bfloat16, tag="om1", name="om1"),
        ]

        def mm_group_kmajor(et, j, s_order=None):
            """k-major matmul group: all NS banks accumulate over k."""
            order = list(s_order) if s_order is not None else list(range(NS))
            ps = [
                psum.tile([P, SCH], dt.float32, tag=f"p{s}", name=f"ps{s}_{et}_{j}")
                for s in range(NS)
            ]
            for k in range(KT):
                lhsT = wop(et, j, k)
                for s in order:
                    nc.tensor.matmul(
                        ps[s][:], lhsT, xop(k, s),
                        start=(k == 0), stop=(k == KT - 1),
                    )
            return ps



        def mm_bank_smajor(et, j, s, tag, name):
            """One s-bank accumulated over all k (s-major building block)."""
            t = psum.tile([P, SCH], dt.float32, tag=tag, name=name)
            for k in range(KT):
                nc.tensor.matmul(
                    t[:], wop(et, j, k), xop(k, s),
                    start=(k == 0), stop=(k == KT - 1),
                )
            return t

        def evict_sigmoid(ps, dst, et, j):
            bcol = btile[:, j * ET + et: j * ET + et + 1]
            for s in range(NS):
                sl = slice(s * SCH, (s + 1) * SCH)
                nc.scalar.activation(dst[:, sl], ps[s][:], AF.Sigmoid, bias=bcol)

        def evict_u(ps):
            for s in range(NS):
                sl = slice(s * SCH, (s + 1) * SCH)
                nc.vector.scalar_tensor_tensor(
                    u[:, sl], a[:, sl], 1.0, ps[s][:],
                    op0=OP.subtract, op1=OP.mult,
                )

        def scan_mult_store(et, g, om):
            nbv = btile[:, 3 * ET + et: 3 * ET + et + 1]   # -bv
            pbv = btile[:, 1 * ET + et: 1 * ET + et + 1]   # +bv
            # h'_t = a_t * h'_{t-1} - u'_t, h'_{-1} = -bv
            nc.vector.tensor_tensor_scan(
                h[:], a[:], u[:], nbv, op0=OP.mult, op1=OP.subtract
            )
            # out = (h' + bv) * g
            nc.vector.scalar_tensor_tensor(
                om[:], h[:], pbv, g[:], op0=OP.add, op1=OP.mult
            )
            nc.sync.dma_start(out[et * P:(et + 1) * P, :], om[:])

        # ---- e-tile 0: decay+gate interleaved s-major (PE consumes x at
        # half rate so the streaming x slices keep up), then value k-major.
        g = gbuf[0]
        bd_col = btile[:, 0 * ET + 0: 0 * ET + 1]
        bg_col = btile[:, 2 * ET + 0: 2 * ET + 1]
        for s in range(NS):
            sl = slice(s * SCH, (s + 1) * SCH)
            pa = mm_bank_smajor(0, 0, s, f"p{(2 * s) % NS}", f"pa{s}_0")
            nc.scalar.activation(a[:, sl], pa[:], AF.Sigmoid, bias=bd_col)
            pg = mm_bank_smajor(0, 2, s, f"p{(2 * s + 1) % NS}", f"pg{s}_0")
            nc.scalar.activation(g[:, sl], pg[:], AF.Sigmoid, bias=bg_col)
        ps = mm_group_kmajor(0, 1)
        evict_u(ps)
        scan_mult_store(0, g, ombuf[0])
        _load_w(3)

        # ---- e-tiles 1..ET-2: decay, value, gate (k-major).
        for et in range(1, ET - 1):
            g = gbuf[et % 2]
            ps = mm_group_kmajor(et, 0)
            evict_sigmoid(ps, a, et, 0)
            ps = mm_group_kmajor(et, 1)
            evict_u(ps)
            ps = mm_group_kmajor(et, 2)
            evict_sigmoid(ps, g, et, 2)
            scan_mult_store(et, g, ombuf[et % 2])
            if et + 3 < ET:
                _load_w(et + 3)

        # ---- last e-tile: decay (k-major), value (s-major, u-evicts trail
        # each bank, scan in two chained halves behind them), gate (s-major,
        # sigmoids trail each bank), then chunked out-multiplies + stores
        # that trail the gate evictions. Tail after the last matmul is just
        # one eviction + one small multiply + one small store.
        et = ET - 1
        g = gbuf[et % 2]
        om = ombuf[et % 2]
        bd_col = btile[:, 0 * ET + et: 0 * ET + et + 1]
        bg_col = btile[:, 2 * ET + et: 2 * ET + et + 1]
        nbv = btile[:, 3 * ET + et: 3 * ET + et + 1]
        pbv = btile[:, 1 * ET + et: 1 * ET + et + 1]

        ps = mm_group_kmajor(et, 0)
        evict_sigmoid(ps, a, et, 0)
        half = S // 2
        for s in range(NS):
            sl = slice(s * SCH, (s + 1) * SCH)
            pv = mm_bank_smajor(et, 1, s, f"p{s}", f"pv{s}_{et}")
            nc.vector.scalar_tensor_tensor(
                u[:, sl], a[:, sl], 1.0, pv[:], op0=OP.subtract, op1=OP.mult
            )
            if s == NS // 2 - 1:
                nc.vector.tensor_tensor_scan(
                    h[:, 0:half], a[:, 0:half], u[:, 0:half], nbv,
                    op0=OP.mult, op1=OP.subtract,
                )
            elif s == NS - 1:
                nc.vector.tensor_tensor_scan(
                    h[:, half:S], a[:, half:S], u[:, half:S],
                    h[:, half - 1:half], op0=OP.mult, op1=OP.subtract,
                )
        # gate s-major with the out-multiply + store interleaved per bank:
        # earlier chunks drain while later gate banks still stream, so the
        # post-last-matmul chain is only ACT(256) + mult(256) + one small
        # store. The final bank is built as two 256-col accumulation
        # groups in the same psum bank (sequential groups; the second
        # group's writes never touch the first half's bytes).
        for s in range(NS):
            sl = slice(s * SCH, (s + 1) * SCH)
            if s < NS - 1:
                pg = mm_bank_smajor(et, 2, s, f"p{s}", f"pg{s}_{et}")
                nc.scalar.activation(g[:, sl], pg[:], AF.Sigmoid, bias=bg_col)
                nc.vector.scalar_tensor_tensor(
                    om[:, sl], h[:, sl], pbv, g[:, sl],
                    op0=OP.add, op1=OP.mult,
                )
                nc.sync.dma_start(out[et * P:(et + 1) * P, sl], om[:, sl])
            else:
                t = psum.tile([P, SCH], dt.float32, tag=f"p{s}", name=f"pg{s}_{et}")
                for hf in range(2):
                    hsl = slice(s * SCH + hf * 256, s * SCH + hf * 256 + 256)
                    pslice = t[:, hf * 256:hf * 256 + 256]
                    for k in range(KT):
                        nc.tensor.matmul(
                            pslice, wop(et, 2, k),
                            xop(k, s)[:, hf * 256:hf * 256 + 256],
                            start=(k == 0), stop=(k == KT - 1),
                        )
                    nc.scalar.activation(
                        g[:, hsl], pslice, AF.Sigmoid, bias=bg_col
                    )
                    nc.vector.scalar_tensor_tensor(
                        om[:, hsl], h[:, hsl], pbv, g[:, hsl],
                        op0=OP.add, op1=OP.mult,
                    )
                    nc.sync.dma_start(out[et * P:(et + 1) * P, hsl], om[:, hsl])

    nc.compile()
    _nc_cache[key] = nc
    return nc


def _start_trace():
    """Begin an NRT/NTFF profile capture on core 0 via the axon PJRT .so.

    Dev-only (MINGRU_TRACE=1); returns None on any failure so the normal
    execution path is never affected.
    """
    try:
        import ctypes
        import tempfile

        so = "/opt/axon/libaxon_pjrt.so"
        lib = ctypes.CDLL(so)
        if not hasattr(lib, "axon_start_nrt_profile"):
            return None
        lib.axon_start_nrt_profile.argtypes = [
            ctypes.POINTER(ctypes.c_int64),
            ctypes.c_size_t,
        ]
        lib.axon_start_nrt_profile.restype = ctypes.c_int64
        lib.axon_stop_nrt_profile.argtypes = [ctypes.c_char_p]
        lib.axon_stop_nrt_profile.restype = ctypes.c_int64

        import jax

        jax.devices()
        ids = (ctypes.c_int64 * 1)(0)
        rc = lib.axon_start_nrt_profile(ids, 1)
        if rc != 0:
            print(f"trace: axon_start_nrt_profile rc={rc}")
            return None
        outdir = tempfile.mkdtemp(prefix="mingru_ntff_")
        return (lib, outdir)
    except Exception as e:
        print(f"trace: start failed: {e!r}")
        return None


def _stop_trace(tracer, nc):
    """Stop the capture, convert NTFF -> perfetto, stash BassKernelResults."""
    lib, outdir = tracer
    try:
        n = lib.axon_stop_nrt_profile(str(outdir).encode())
        print(f"trace: {n} file(s) written to {outdir}")
        if n <= 0:
            return
        import gauge.profiler
        from concourse import bass_utils
        from concourse._compat import FishPath

        profile = gauge.profiler.Profile(
            profile_path=FishPath(outdir),
            kernel_dev_mode=True,
            profile_on_exit=False,
            bass_kernel=nc.m,
            offline_processing=True,
            fname="*_body*",
            metadata={},
        )
        perf = bass_utils._process_ntff_profile(
            profile,
            outdir,
            nc,
            core_ids=list(range(B)),
            trace_cores=[0],
            stitch_traces=False,
            trace_kwargs={},
            trace_events=False,
        )
        _nc_cache["last_results"] = perf.as_bass_kernel_results([])
    except Exception as e:
        print(f"trace: postprocess failed: {e!r}")


def _run_spmd_sharded(nc, in_maps, n_cores):
    """Like bass2jax.run_bass_via_pjrt, but moves data per-shard (16MB max per
    transfer) instead of one big concatenated host<->device transfer, which
    overflows the axon tunnel at our sizes (128MB outputs)."""
    import jax
    import jax.numpy as jnp
    import concourse.mybir as mybir
    from concourse import bass2jax
    from jax.sharding import Mesh, NamedSharding, PartitionSpec
    from jax.experimental.shard_map import shard_map

    bass2jax.install_neuronx_cc_hook()

    partition_name = nc.partition_id_tensor.name if nc.partition_id_tensor else None

    in_names, out_names, out_avals = [], [], []
    for alloc in nc.m.functions[0].allocations:
        if not isinstance(alloc, mybir.MemoryLocationSet):
            continue
        name = alloc.memorylocations[0].name
        if alloc.kind == "ExternalInput":
            if name != partition_name:
                in_names.append(name)
        elif alloc.kind == "ExternalOutput":
            out_names.append(name)
            out_avals.append(
                jax.core.ShapedArray(
                    tuple(alloc.tensor_shape), mybir.dt.np(alloc.dtype)
                )
            )
    n_params = len(in_names)
    n_outs = len(out_avals)
    in_names = in_names + out_names
    if partition_name is not None:
        in_names.append(partition_name)
    donate = tuple(range(n_params, n_params + n_outs))

    def _body(*args):
        operands = list(args)
        if partition_name is not None:
            operands.append(bass2jax.partition_id_tensor())
        return tuple(
            bass2jax._bass_exec_p.bind(
                *operands,
                out_avals=tuple(out_avals),
                in_names=tuple(in_names),
                out_names=tuple(out_names),
                lowering_input_output_aliases=(),
                sim_require_finite=True,
                sim_require_nnan=True,
                nc=nc,
            )
        )

    devices = jax.devices()[:n_cores]
    mesh = Mesh(np.asarray(devices), ("core",))
    sharding = NamedSharding(mesh, PartitionSpec("core"))
    in_specs = (PartitionSpec("core"),) * (n_params + n_outs)
    out_specs = (PartitionSpec("core"),) * n_outs
    fn = jax.jit(
        shard_map(
            _body, mesh=mesh, in_specs=in_specs, out_specs=out_specs, check_rep=False
        ),
        donate_argnums=donate,
        keep_unused=True,
    )

    def put(name):
        shards = [
            jax.device_put(np.asarray(in_maps[c][name]), devices[c])
            for c in range(n_cores)
        ]
        shp = shards[0].shape
        return jax.make_array_from_single_device_arrays(
            (n_cores * shp[0], *shp[1:]), sharding, shards
        )

    args = [put(name) for name in in_names[:n_params]]
    zeros = [
        jnp.zeros((n_cores * a.shape[0], *a.shape[1:]), a.dtype, device=sharding)
        for a in out_avals
    ]

    tracer = _start_trace() if os.environ.get("MINGRU_TRACE") == "1" else None
    out_arrs = fn(*args, *zeros)
    jax.block_until_ready(out_arrs)
    if tracer is not None:
        _stop_trace(tracer, nc)

    results = [dict() for _ in range(n_cores)]
    for i, name in enumerate(out_names):
        shards = sorted(
            out_arrs[i].addressable_shards, key=lambda s: s.index[0].start or 0
        )
        assert len(shards) == n_cores
        for c in range(n_cores):
            results[c][name] = np.asarray(shards[c].data)
    return results


def kernel(x, Wg, bg, Wv, bv, Wd, bd):
    x = np.asarray(x)
    nc = _build_nc(bv_zero=not np.any(np.asarray(bv)))

    # Pack weights per (et, j): wt[et, j, p, k*P+e] = W_j.T[k*P+p, et*P+e].
    # Weights are pre-divided by 2 (exact) to compensate the x*2 pre-scale
    # applied before the fp8 E3M4 cast (uses e3m4's range/subnormal zone
    # better; x*2 stays within its +-15.5 max).
    def pack(w):
        wT = np.ascontiguousarray(np.asarray(w, np.float32).T * 0.5)  # [d, e]
        return (
            wT.reshape(KT, P, ET, P)
            .transpose(2, 1, 0, 3)
            .reshape(ET, P, KT * P)
        )

    # stack j on the last-but-one axis then flatten: [ET, P, 3*KT*P]
    wt = np.ascontiguousarray(
        np.stack([pack(Wd), pack(Wv), pack(Wg)], axis=2).reshape(
            ET, P, 3 * KT * P
        )
    ).astype(_BF16)
    bv = np.asarray(bv)
    # bias packed to [P, 4*ET]: col j*ET+e holds bias_j[e*P+p] (contiguous
    # rows -> cheap DMA descriptor generation).
    bias = np.ascontiguousarray(
        np.stack([np.asarray(bd), bv, np.asarray(bg), -bv])
        .astype(np.float32)
        .reshape(4, ET, P)
        .transpose(2, 0, 1)
        .reshape(P, 4 * ET)
    )

    in_maps = []
    for b in range(B):
        xtr = np.ascontiguousarray(
            np.asarray(x[b].T, np.float32) * 2.0
        ).astype(_E3M4)  # [D, S] fp8 e3m4, pre-scaled by 2
        in_maps.append({"xt": xtr, "wt": wt, "bias": bias})

    results = _run_spmd_sharded(nc, in_maps, n_cores=B)

    out = np.empty((B, S, D), np.float32)
    for b in range(B):
        out[b] = results[b]["out"].T.astype(np.float32)
    return out



# revision 26
# speedup vs baseline: 1.0010x; 1.0010x over previous
"""MinGRU layer on 8 Trainium2 NeuronCores (batch-parallel).

Math (per batch b, reference semantics, all fp32):
    g = sigmoid(x @ Wg.T + bg)
    a = sigmoid(x @ Wd.T + bd)
    v = x @ Wv.T + bv
    h_t = a_t * h_{t-1} + (1 - a_t) * v_t     (causal scan over S)
    out = h * g

Design (measured-bottleneck-driven; PE matmul work is ~332us of the ~354us
total, i.e. the kernel runs at ~95% of the bf16 tensor-engine roofline;
steady-state pass gap is exactly 512 PE cycles, zero per-instruction
overhead):

Measured dead ends (don't retry):
  - fp8 DoubleRow runs 512-col passes at the SAME 216ns as bf16 (2x FLOPs
    via K=256/pass, matching the 157TF/s spec; the CoreSim cost model's
    0.5 cycles/row = 4x is wrong on hardware). Precision needs both
    operands split hi+lo (12 passes/proj vs bf16's 8) => 1.5x SLOWER.
    Plain fp8 e4m3 rel-err is 5.3e-2 > 2e-2 tolerance; single-operand
    splits 3-4e-2. fp8 is strictly dominated here.
  - 1024-col matmuls (2 psum banks) are ISA-illegal (walrus NCC_IXCG864).
  - Starting the PE earlier on finer first-x slices loses ~2us: the
    e-tile-0 phase is x-supply-bound (~390GB/s effective, stream done
    ~40us), so an earlier start converts free front idle into mid-stream
    stall events.
  - ~30 periodic +216ns PE stalls at a fixed ~10.8us cadence (~7us) are
    system-level (present in pure-PE microbenches; unfixable).
  - Whole runs execute at a per-run DVFS state (216/235/259 ns per pass =
    2.37/2.18/1.98 GHz), flat for the entire run: measured exec varies
    354k..421k ns for identical code. Compare runs by median pass gap.
  - B=8 batches -> one batch element per NeuronCore (embarrassingly parallel).
  - Host side: x[b] -> X.T [D, S] bf16; weights packed per e-tile into
    [P, 3*KT*P] rows (one contiguous 6KB DMA row per partition); bias packed
    to [P, 4*ET]; all to minimize DMA instruction count (each dma_start costs
    ~0.7us of serialized dispatch on the sync queue — the startup gate).
  - On-chip per core:
      * matmuls produce projections directly in [e(part), s(free)] layout:
        out[e, s] = sum_d W.T[d, e] * X.T[d, s]  via PE (stationary = W.T tile)
      * ACT evicts PSUM with fused bias+sigmoid (gate/decay); DVE evicts the
        value projection as u' = (a-1)*v straight out of PSUM
      * DVE runs the recurrence via the hardware TensorTensorScanArith op:
        state = a*state - u' = a*state + (1-a)*v  (bv folded via init -bv)
      * out = (h'+bv)*g -> bf16, DMA back to DRAM in [e, s]; host transposes
        and upcasts (bf16 halves the final HBM write drain).
  - Schedule: ~110 junk warm-up matmuls burn the TRN2 PE p-state ramp
    (0.65->2.4GHz over ~3us busy) while the first x slices stream in; the
    first e-tile interleaves decay+gate s-major so PE consumption matches
    the x arrival rate; middle e-tiles run decay/value/gate k-major with
    full-tile scans overlapped one group behind; the last e-tile runs
    decay, value s-major (u-evicts + two chained scan halves trail the
    banks), then gate s-major with the out-multiply + per-bank store
    interleaved so earlier chunks drain while later banks stream; the
    final gate bank is two 256-col accumulation groups in one psum bank,
    leaving only ACT(256)+mult(256)+one small store after the last
    matmul (exec ends ~6.3-6.8us after the last matmul; ~4.4us of that
    is the framework sem-teardown epilogue).
  - g/om work tiles are parity-double-buffered to break WAR stalls between
    consecutive e-tiles; weight slabs rotate 3 tags deep, loaded ~2 e-tiles
    ahead.
"""

import os

import numpy as np
import ml_dtypes

B, S, D = 8, 4096, 1024
P = 128
KT = D // P          # 8 contraction tiles
ET = D // P          # 8 output-channel tiles
SCH = 512            # psum free-dim chunk (one bank, fp32)
NS = S // SCH        # 8 s-chunks
XC = 1024            # x DMA column-chunk
NXC = S // XC

_BF16 = ml_dtypes.bfloat16
_E3M4 = ml_dtypes.float8_e3m4

_nc_cache = {}


def _build_nc(bv_zero=True):
    """Build + compile the single-core Bass program (shared by all 8 cores)."""
    key = ("nc", bv_zero)
    if key in _nc_cache:
        return _nc_cache[key]

    from contextlib import ExitStack

    import concourse.bacc as bacc
    import concourse.mybir as mybir
    from concourse import tile

    dt = mybir.dt
    AF = mybir.ActivationFunctionType
    OP = mybir.AluOpType

    nc = bacc.Bacc("TRN2", target_bir_lowering=False, debug=False, num_devices=8)

    # x ships as fp8 E3M4 (5 significant bits), pre-scaled by 2 on the host
    # with the weights pre-divided by 2 (exact exponent shifts) so the
    # products are unchanged. E3M4 is a full-rate matmul moving operand
    # (1.0 cycles/row, same 216ns/pass) and HALVES the x stream (8.4->4.2
    # MB), which gates the startup. Simulated end-to-end error (simulator
    # matches HW to 7 digits on the bf16 config): 1.74e-2 < 2e-2 tol.
    xt = nc.dram_tensor("xt", [D, S], dt.float8e3, kind="ExternalInput").ap()
    # wt[et, p, j*KT*P + k*P + e] = W_j.T[k*P+p, et*P+e]: one contiguous 6KB
    # row per partition per e-tile -> a single cheap DMA per e-tile.
    # j: 0=decay(Wd), 1=value(Wv), 2=gate(Wg)
    wt = nc.dram_tensor("wt", [ET, P, 3 * KT * P], dt.bfloat16, kind="ExternalInput").ap()
    # bias pre-packed host-side to [P, 4*ET]: col j*ET+e holds bias_j[e*P+p];
    # j: 0=bd, 1=bv, 2=bg, 3=-bv  (bv folded into the scan: h = h'+bv,
    # h' scans (a-1)*(xWv) with initial -bv, out = (h'+bv)*g)
    bias = nc.dram_tensor("bias", [P, 4 * ET], dt.float32, kind="ExternalInput").ap()
    # bf16 output halves the HBM write drain; the host upcasts. Rounding adds
    # <= 0.2% of |out| — far inside the tolerance.
    out = nc.dram_tensor("out", [D, S], dt.bfloat16, kind="ExternalOutput").ap()

    with tile.TileContext(nc) as tc, ExitStack() as ctx:
        xp = ctx.enter_context(tc.tile_pool(name="xp", bufs=1))
        wp = ctx.enter_context(tc.tile_pool(name="wp", bufs=1))
        bp = ctx.enter_context(tc.tile_pool(name="bp", bufs=1))
        work = ctx.enter_context(tc.tile_pool(name="work", bufs=1))
        psum = ctx.enter_context(tc.tile_pool(name="psum", bufs=1, space="PSUM"))

        # Weight slabs rotate through 3 e-tiles' worth of tags; one DMA per
        # e-tile loads all three projections (3 x 2KB rows per partition).
        wtiles = {}

        def _load_w(et, j=None):
            """Load e-tile et's weights; j=None loads all three projections
            in one DMA, j=int loads only that projection's slab (used for
            e-tile 0 so the first matmuls wait on a 256KB transfer, not
            the full 768KB)."""
            if et in wtiles:
                t = wtiles[et]
            else:
                t = wp.tile(
                    [P, 3 * KT * P], dt.bfloat16, tag=f"wt{et % 3}", name=f"w{et}"
                )
                wtiles[et] = t
            if j is None:
                nc.sync.dma_start(t[:], wt[et])
            else:
                sl = slice(j * KT * P, (j + 1) * KT * P)
                nc.sync.dma_start(t[:, sl], wt[et][:, sl])

        def wop(et, j, k):
            return wtiles[et][:, j * KT * P + k * P:(j * KT * P + k * P) + P]

        # x: column-slice tiles, each holding a column range for a k-tile
        # range (one DMA instruction covers many matmul operands; per-DMA
        # dispatch on the sync queue is ~0.6us, the startup gate). The first
        # 512 columns are split into two k-halves so the very first matmuls
        # have a small (512KB) transfer to wait on. (Starting the PE earlier
        # on finer slices was measured to LOSE ~2us: the e-tile-0 phase is
        # x-supply-bound, so an earlier start just converts free front idle
        # into mid-stream stalls with extra restart overhead.)
        XSL = [(0, 4, 0, 512), (4, 8, 0, 512), (0, 4, 512, 1024),
               (4, 8, 512, 1024), (0, 4, 1024, 2048), (4, 8, 1024, 2048),
               (0, 4, 2048, 3072), (4, 8, 2048, 3072), (0, 4, 3072, 4096),
               (4, 8, 3072, 4096)]
        xtile = [None] * len(XSL)
        xt3 = xt.rearrange("(k p) s -> p k s", p=P)

        def _load_x(i):
            k0, k1, c0, c1 = XSL[i]
            t = xp.tile(
                [P, (k1 - k0) * (c1 - c0)], dt.float8e3, tag=f"x{i}", name=f"x{i}"
            )
            nc.sync.dma_start(
                t[:].rearrange("p (k s) -> p k s", k=k1 - k0),
                xt3[:, k0:k1, c0:c1],
            )
            xtile[i] = t

        def xop(k, s):
            c = s * SCH
            for i, (k0, k1, c0, c1) in enumerate(XSL):
                if k0 <= k < k1 and c0 <= c < c1:
                    o = (k - k0) * (c1 - c0) + (c - c0)
                    return xtile[i][:, o:o + SCH]
            raise AssertionError

        # PE warm-up: TRN2 ramps the PE clock 0.65 -> 2.4 GHz over ~3us of
        # busy time. Burn the ramp on junk matmuls over a memset tile while
        # the input DMAs are still in flight (~5us; ends about when the
        # first x slices land).
        warm = bp.tile([P, 64], dt.bfloat16, name="warm")
        nc.gpsimd.memset(warm[:], 0.0)
        wps = psum.tile([64, 64], dt.float32, tag="p0", name="wps")
        for r in range(110):
            nc.tensor.matmul(wps[:], warm[:], warm[:], start=True, stop=True)

        # DMA issue order ~ first-use order; the bytes ahead of the first
        # matmul's operands gate the pipeline start. With x at fp8 the
        # stream has 2x supply slack, so the per-projection w0 slab split
        # (which LOST under bf16 x by creating supply-pacing stalls) now
        # pays: only 256KB of weights + 512KB of x gate the first matmul.
        _load_w(0, j=0)
        _load_x(0)
        _load_x(1)
        btile = bp.tile([P, 4 * ET], dt.float32)
        nc.sync.dma_start(btile[:], bias)
        # Dummy sigmoid hoists the ACT table load to kernel start.
        scratch = bp.tile([P, 1], dt.float32, name="scratch")
        nc.scalar.activation(
            scratch[:], btile[:, 0:1], AF.Sigmoid, bias=btile[:, 1:2]
        )
        _load_w(0, j=2)
        _load_x(2)
        _load_x(3)
        _load_w(0, j=1)
        for i in range(4, len(XSL)):
            _load_x(i)
        _load_w(1)
        _load_w(2)

        # Work tiles: g and om are double-buffered (parity) to break WAR
        # stalls against the previous e-tile's consumers.
        a = work.tile([P, S], dt.float32, tag="a", name="a")
        u = work.tile([P, S], dt.float32, tag="u", name="u")
        h = work.tile([P, S], dt.float32, tag="h", name="h")
        gbuf = [
            work.tile([P, S], dt.float32, tag="g0", name="g0"),
            work.tile([P, S], dt.float32, tag="g1", name="g1"),
        ]
        ombuf = [
            work.tile([P, S], dt.bfloat16, tag="om0", name="om0"),
            work.tile([P, S], dt.bfloat16, tag="om1", name="om1"),
        ]

        def mm_group_kmajor(et, j, s_order=None):
            """k-major matmul group: all NS banks accumulate over k."""
            order = list(s_order) if s_order is not None else list(range(NS))
            ps = [
                psum.tile([P, SCH], dt.float32, tag=f"p{s}", name=f"ps{s}_{et}_{j}")
                for s in range(NS)
            ]
            for k in range(KT):
                lhsT = wop(et, j, k)
                for s in order:
                    nc.tensor.matmul(
                        ps[s][:], lhsT, xop(k, s),
                        start=(k == 0), stop=(k == KT - 1),
                    )
            return ps



        def mm_bank_smajor(et, j, s, tag, name):
            """One s-bank accumulated over all k (s-major building block)."""
            t = psum.tile([P, SCH], dt.float32, tag=tag, name=name)
            for k in range(KT):
                nc.tensor.matmul(
                    t[:], wop(et, j, k), xop(k, s),
                    start=(k == 0), stop=(k == KT - 1),
                )
            return t

        def evict_sigmoid(ps, dst, et, j):
            bcol = btile[:, j * ET + et: j * ET + et + 1]
            for s in range(NS):
                sl = slice(s * SCH, (s + 1) * SCH)
                nc.scalar.activation(dst[:, sl], ps[s][:], AF.Sigmoid, bias=bcol)

        def evict_u(ps):
            for s in range(NS):
                sl = slice(s * SCH, (s + 1) * SCH)
                nc.vector.scalar_tensor_tensor(
                    u[:, sl], a[:, sl], 1.0, ps[s][:],
                    op0=OP.subtract, op1=OP.mult,
                )

        def scan_mult_store(et, g, om):
            nbv = btile[:, 3 * ET + et: 3 * ET + et + 1]   # -bv
            pbv = btile[:, 1 * ET + et: 1 * ET + et + 1]   # +bv
            # h'_t = a_t * h'_{t-1} - u'_t, h'_{-1} = -bv
            nc.vector.tensor_tensor_scan(
                h[:], a[:], u[:], nbv, op0=OP.mult, op1=OP.subtract
            )
            # out = (h' + bv) * g
            nc.vector.scalar_tensor_tensor(
                om[:], h[:], pbv, g[:], op0=OP.add, op1=OP.mult
            )
            nc.sync.dma_start(out[et * P:(et + 1) * P, :], om[:])

        # ---- e-tile 0: decay+gate interleaved s-major (PE consumes x at
        # half rate so the streaming x slices keep up), then value k-major.
        g = gbuf[0]
        bd_col = btile[:, 0 * ET + 0: 0 * ET + 1]
        bg_col = btile[:, 2 * ET + 0: 2 * ET + 1]
        for s in range(NS):
            sl = slice(s * SCH, (s + 1) * SCH)
            pa = mm_bank_smajor(0, 0, s, f"p{(2 * s) % NS}", f"pa{s}_0")
            nc.scalar.activation(a[:, sl], pa[:], AF.Sigmoid, bias=bd_col)
            pg = mm_bank_smajor(0, 2, s, f"p{(2 * s + 1) % NS}", f"pg{s}_0")
            nc.scalar.activation(g[:, sl], pg[:], AF.Sigmoid, bias=bg_col)
        ps = mm_group_kmajor(0, 1)
        evict_u(ps)
        scan_mult_store(0, g, ombuf[0])
        _load_w(3)

        # ---- e-tiles 1..ET-2: decay, value, gate (k-major).
        for et in range(1, ET - 1):
            g = gbuf[et % 2]
            ps = mm_group_kmajor(et, 0)
            evict_sigmoid(ps, a, et, 0)
            ps = mm_group_kmajor(et, 1)
            evict_u(ps)
            ps = mm_group_kmajor(et, 2)
            evict_sigmoid(ps, g, et, 2)
            scan_mult_store(et, g, ombuf[et % 2])
            if et + 3 < ET:
                _load_w(et + 3)

        # ---- last e-tile: decay (k-major), value (s-major, u-evicts trail
        # each bank, scan in two chained halves behind them), gate (s-major,
        # sigmoids trail each bank), then chunked out-multiplies + stores
        # that trail the gate evictions. Tail after the last matmul is just
        # one eviction + one small multiply + one small store.
        et = ET - 1
        g = gbuf[et % 2]
        om = ombuf[et % 2]
        bd_col = btile[:, 0 * ET + et: 0 * ET + et + 1]
        bg_col = btile[:, 2 * ET + et: 2 * ET + et + 1]
        nbv = btile[:, 3 * ET + et: 3 * ET + et + 1]
        pbv = btile[:, 1 * ET + et: 1 * ET + et + 1]

        ps = mm_group_kmajor(et, 0)
        evict_sigmoid(ps, a, et, 0)
        half = S // 2
        for s in range(NS):
            sl = slice(s * SCH, (s + 1) * SCH)
            pv = mm_bank_smajor(et, 1, s, f"p{s}", f"pv{s}_{et}")
            nc.vector.scalar_tensor_tensor(
                u[:, sl], a[:, sl], 1.0, pv[:], op0=OP.subtract, op1=OP.mult
            )
            if s == NS // 2 - 1:
                nc.vector.tensor_tensor_scan(
                    h[:, 0:half], a[:, 0:half], u[:, 0:half], nbv,
                    op0=OP.mult, op1=OP.subtract,
                )
            elif s == NS - 1:
                nc.vector.tensor_tensor_scan(
                    h[:, half:S], a[:, half:S], u[:, half:S],
                    h[:, half - 1:half], op0=OP.mult, op1=OP.subtract,
                )
        # gate s-major with the out-multiply + store interleaved per bank:
        # earlier chunks drain while later gate banks still stream, so the
        # post-last-matmul chain is only ACT(256) + mult(256) + one small
        # store. The final bank is built as two 256-col accumulation
        # groups in the same psum bank (sequential groups; the second
        # group's writes never touch the first half's bytes).
        for s in range(NS):
            sl = slice(s * SCH, (s + 1) * SCH)
            if s < NS - 1:
                pg = mm_bank_smajor(et, 2, s, f"p{s}", f"pg{s}_{et}")
                nc.scalar.activation(g[:, sl], pg[:], AF.Sigmoid, bias=bg_col)
                nc.vector.scalar_tensor_tensor(
                    om[:, sl], h[:, sl], pbv, g[:, sl],
                    op0=OP.add, op1=OP.mult,
                )
                nc.sync.dma_start(out[et * P:(et + 1) * P, sl], om[:, sl])
            else:
                t = psum.tile([P, SCH], dt.float32, tag=f"p{s}", name=f"pg{s}_{et}")
                for hf in range(2):
                    hsl = slice(s * SCH + hf * 256, s * SCH + hf * 256 + 256)
                    pslice = t[:, hf * 256:hf * 256 + 256]
                    for k in range(KT):
                        nc.tensor.matmul(
                            pslice, wop(et, 2, k),
                            xop(k, s)[:, hf * 256:hf * 256 + 256],
                            start=(k == 0), stop=(k == KT - 1),
                        )
                    nc.scalar.activation(
                        g[:, hsl], pslice, AF.Sigmoid, bias=bg_col
                    )
                    nc.vector.scalar_tensor_tensor(
                        om[:, hsl], h[:, hsl], pbv, g[:, hsl],
                        op0=OP.add, op1=OP.mult,
                    )
                    nc.sync.dma_start(out[et * P:(et + 1) * P, hsl], om[:, hsl])

    nc.compile()
    _nc_cache[key] = nc
    return nc


def _start_trace():
    """Begin an NRT/NTFF profile capture on core 0 via the axon PJRT .so.

    Dev-only (MINGRU_TRACE=1); returns None on any failure so the normal
    execution path is never affected.
    """
    try:
        import ctypes
        import tempfile

        so = "/opt/axon/libaxon_pjrt.so"
        lib = ctypes.CDLL(so)
        if not hasattr(lib, "axon_start_nrt_profile"):
            return None
        lib.axon_start_nrt_profile.argtypes = [
            ctypes.POINTER(ctypes.c_int64),
            ctypes.c_size_t,
        ]
        lib.axon_start_nrt_profile.restype = ctypes.c_int64
        lib.axon_stop_nrt_profile.argtypes = [ctypes.c_char_p]
        lib.axon_stop_nrt_profile.restype = ctypes.c_int64

        import jax

        jax.devices()
        ids = (ctypes.c_int64 * 1)(0)
        rc = lib.axon_start_nrt_profile(ids, 1)
        if rc != 0:
            print(f"trace: axon_start_nrt_profile rc={rc}")
            return None
        outdir = tempfile.mkdtemp(prefix="mingru_ntff_")
        return (lib, outdir)
    except Exception as e:
        print(f"trace: start failed: {e!r}")
        return None


def _stop_trace(tracer, nc):
    """Stop the capture, convert NTFF -> perfetto, stash BassKernelResults."""
    lib, outdir = tracer
    try:
        n = lib.axon_stop_nrt_profile(str(outdir).encode())
        print(f"trace: {n} file(s) written to {outdir}")
        if n <= 0:
            return
        import gauge.profiler
        from concourse import bass_utils
        from concourse._compat import FishPath

        profile = gauge.profiler.Profile(
            profile_path=FishPath(outdir),
            kernel_dev_mode=True,
            profile_on_exit=False,
            bass_kernel=nc.m,
            offline_processing=True,
            fname="*_body*",
            metadata={},
        )
        perf = bass_utils._process_ntff_profile(
            profile,
            outdir,
            nc,
            core_ids=list(range(B)),
            trace_cores=[0],
            stitch_traces=False,
            trace_kwargs={},
            trace_events=False,
        )
        _nc_cache["last_results"] = perf.as_bass_kernel_results([])
    except Exception as e:
        print(f"trace: postprocess failed: {e!r}")


def _run_spmd_sharded(nc, in_maps, n_cores):
    """Like bass2jax.run_bass_via_pjrt, but moves data per-shard (16MB max per
    transfer) instead of one big concatenated host<->device transfer, which
    overflows the axon tunnel at our sizes (128MB outputs)."""
    import jax
    import jax.numpy as jnp
    import concourse.mybir as mybir
    from concourse import bass2jax
    from jax.sharding import Mesh, NamedSharding, PartitionSpec
    from jax.experimental.shard_map import shard_map

    bass2jax.install_neuronx_cc_hook()

    partition_name = nc.partition_id_tensor.name if nc.partition_id_tensor else None

    in_names, out_names, out_avals = [], [], []
    for alloc in nc.m.functions[0].allocations:
        if not isinstance(alloc, mybir.MemoryLocationSet):
            continue
        name = alloc.memorylocations[0].name
        if alloc.kind == "ExternalInput":
            if name != partition_name:
                in_names.append(name)
        elif alloc.kind == "ExternalOutput":
            out_names.append(name)
            out_avals.append(
                jax.core.ShapedArray(
                    tuple(alloc.tensor_shape), mybir.dt.np(alloc.dtype)
                )
            )
    n_params = len(in_names)
    n_outs = len(out_avals)
    in_names = in_names + out_names
    if partition_name is not None:
        in_names.append(partition_name)
    donate = tuple(range(n_params, n_params + n_outs))

    def _body(*args):
        operands = list(args)
        if partition_name is not None:
            operands.append(bass2jax.partition_id_tensor())
        return tuple(
            bass2jax._bass_exec_p.bind(
                *operands,
                out_avals=tuple(out_avals),
                in_names=tuple(in_names),
                out_names=tuple(out_names),
                lowering_input_output_aliases=(),
                sim_require_finite=True,
                sim_require_nnan=True,
                nc=nc,
            )
        )

    devices = jax.devices()[:n_cores]
    mesh = Mesh(np.asarray(devices), ("core",))
    sharding = NamedSharding(mesh, PartitionSpec("core"))
    in_specs = (PartitionSpec("core"),) * (n_params + n_outs)
    out_specs = (PartitionSpec("core"),) * n_outs
    fn = jax.jit(
        shard_map(
            _body, mesh=mesh, in_specs=in_specs, out_specs=out_specs, check_rep=False
        ),
        donate_argnums=donate,
        keep_unused=True,
    )

    def put(name):
        shards = [
            jax.device_put(np.asarray(in_maps[c][name]), devices[c])
            for c in range(n_cores)
        ]
        shp = shards[0].shape
        return jax.make_array_from_single_device_arrays(
            (n_cores * shp[0], *shp[1:]), sharding, shards
        )

    args = [put(name) for name in in_names[:n_params]]
    zeros = [
        jnp.zeros((n_cores * a.shape[0], *a.shape[1:]), a.dtype, device=sharding)
        for a in out_avals
    ]

    tracer = _start_trace() if os.environ.get("MINGRU_TRACE") == "1" else None
    out_arrs = fn(*args, *zeros)
    jax.block_until_ready(out_arrs)
    if tracer is not None:
        _stop_trace(tracer, nc)

    results = [dict() for _ in range(n_cores)]
    for i, name in enumerate(out_names):
        shards = sorted(
            out_arrs[i].addressable_shards, key=lambda s: s.index[0].start or 0
        )
        assert len(shards) == n_cores
        for c in range(n_cores):
            results[c][name] = np.asarray(shards[c].data)
    return results


def kernel(x, Wg, bg, Wv, bv, Wd, bd):
    x = np.asarray(x)
    nc = _build_nc(bv_zero=not np.any(np.asarray(bv)))

    # Pack weights per (et, j): wt[et, j, p, k*P+e] = W_j.T[k*P+p, et*P+e].
    # Weights are pre-divided by 2 (exact) to compensate the x*2 pre-scale
    # applied before the fp8 E3M4 cast (uses e3m4's range/subnormal zone
    # better; x*2 stays within its +-15.5 max).
    def pack(w):
        wT = np.ascontiguousarray(np.asarray(w, np.float32).T * 0.5)  # [d, e]
        return (
            wT.reshape(KT, P, ET, P)
            .transpose(2, 1, 0, 3)
            .reshape(ET, P, KT * P)
        )

    # stack j on the last-but-one axis then flatten: [ET, P, 3*KT*P]
    wt = np.ascontiguousarray(
        np.stack([pack(Wd), pack(Wv), pack(Wg)], axis=2).reshape(
            ET, P, 3 * KT * P
        )
    ).astype(_BF16)
    bv = np.asarray(bv)
    # bias packed to [P, 4*ET]: col j*ET+e holds bias_j[e*P+p] (contiguous
    # rows -> cheap DMA descriptor generation).
    bias = np.ascontiguousarray(
        np.stack([np.asarray(bd), bv, np.asarray(bg), -bv])
        .astype(np.float32)
        .reshape(4, ET, P)
        .transpose(2, 0, 1)
        .reshape(P, 4 * ET)
    )

    in_maps = []
    for b in range(B):
        xtr = np.ascontiguousarray(
            np.asarray(x[b].T, np.float32) * 2.0
        ).astype(_E3M4)  # [D, S] fp8 e3m4, pre-scaled by 2
        in_maps.append({"xt": xtr, "wt": wt, "bias": bias})

    results = _run_spmd_sharded(nc, in_maps, n_cores=B)

    out = np.empty((B, S, D), np.float32)
    for b in range(B):
        out[b] = results[b]["out"].T.astype(np.float32)
    return out



# revision 27
# speedup vs baseline: 1.1956x; 1.1944x over previous
"""MinGRU layer on 8 Trainium2 NeuronCores (batch-parallel).

Math (per batch b, reference semantics, all fp32):
    g = sigmoid(x @ Wg.T + bg)
    a = sigmoid(x @ Wd.T + bd)
    v = x @ Wv.T + bv
    h_t = a_t * h_{t-1} + (1 - a_t) * v_t     (causal scan over S)
    out = h * g

Design (measured-bottleneck-driven; PE matmul work is ~332us of the ~354us
total, i.e. the kernel runs at ~95% of the bf16 tensor-engine roofline;
steady-state pass gap is exactly 512 PE cycles, zero per-instruction
overhead):

Measured dead ends (don't retry):
  - fp8 DoubleRow runs 512-col passes at the SAME 216ns as bf16 (2x FLOPs
    via K=256/pass, matching the 157TF/s spec; the CoreSim cost model's
    0.5 cycles/row = 4x is wrong on hardware). Precision needs both
    operands split hi+lo (12 passes/proj vs bf16's 8) => 1.5x SLOWER.
    Plain fp8 e4m3 rel-err is 5.3e-2 > 2e-2 tolerance; single-operand
    splits 3-4e-2. fp8 is strictly dominated here.
  - 1024-col matmuls (2 psum banks) are ISA-illegal (walrus NCC_IXCG864).
  - Starting the PE earlier on finer first-x slices loses ~2us: the
    e-tile-0 phase is x-supply-bound (~390GB/s effective, stream done
    ~40us), so an earlier start converts free front idle into mid-stream
    stall events.
  - ~30 periodic +216ns PE stalls at a fixed ~10.8us cadence (~7us) are
    system-level (present in pure-PE microbenches; unfixable).
  - Whole runs execute at a per-run DVFS state (216/235/259 ns per pass =
    2.37/2.18/1.98 GHz), flat for the entire run: measured exec varies
    354k..421k ns for identical code. Compare runs by median pass gap.
  - B=8 batches -> one batch element per NeuronCore (embarrassingly parallel).
  - Host side: x[b] -> X.T [D, S] in fp8 E3M4, pre-scaled by 2 (weights
    pre-divided by 2, an exact exponent shift) — e3m4 is a full-rate matmul
    moving operand (1.0 cy/row) and halves the x stream that gates startup;
    end-to-end error 1.737e-2 (< 2e-2 tol), predicted by a simulator that
    matches HW output to 7 digits on both the bf16 and fp8 configs.
    Weights packed per e-tile into [P, 3*KT*P] bf16 rows (one contiguous
    6KB DMA row per partition); bias packed to [P, 4*ET]; all to minimize
    DMA instruction count (each dma_start costs ~0.7us of serialized
    dispatch on the sync queue).
  - On-chip per core:
      * matmuls produce projections directly in [e(part), s(free)] layout:
        out[e, s] = sum_d W.T[d, e] * X.T[d, s]  via PE (stationary = W.T tile)
      * ACT evicts PSUM with fused bias+sigmoid (gate/decay); DVE evicts the
        value projection as u' = (a-1)*v straight out of PSUM
      * DVE runs the recurrence via the hardware TensorTensorScanArith op:
        state = a*state - u' = a*state + (1-a)*v  (bv folded via init -bv)
      * out = (h'+bv)*g -> bf16, DMA back to DRAM in [e, s]; host transposes
        and upcasts (bf16 halves the final HBM write drain).
  - Schedule: ~110 junk warm-up matmuls burn the TRN2 PE p-state ramp
    (0.65->2.4GHz over ~3us busy) while the first x slices stream in; the
    first e-tile interleaves decay+gate s-major so PE consumption matches
    the x arrival rate; middle e-tiles run decay/value/gate k-major with
    full-tile scans overlapped one group behind; the last e-tile runs
    decay, value s-major (u-evicts + two chained scan halves trail the
    banks), then gate s-major with the out-multiply + per-bank store
    interleaved so earlier chunks drain while later banks stream; the
    final gate bank is two 256-col accumulation groups in one psum bank,
    leaving only ACT(256)+mult(256)+one small store after the last
    matmul (exec ends ~6.3-6.8us after the last matmul; ~4.4us of that
    is the framework sem-teardown epilogue).
  - g/om work tiles are parity-double-buffered to break WAR stalls between
    consecutive e-tiles; weight slabs rotate 3 tags deep, loaded ~2 e-tiles
    ahead.
"""

import os

import numpy as np
import ml_dtypes

B, S, D = 8, 4096, 1024
P = 128
KT = D // P          # 8 contraction tiles
ET = D // P          # 8 output-channel tiles
SCH = 512            # psum free-dim chunk (one bank, fp32)
NS = S // SCH        # 8 s-chunks
XC = 1024            # x DMA column-chunk
NXC = S // XC

_BF16 = ml_dtypes.bfloat16
_E3M4 = ml_dtypes.float8_e3m4

_nc_cache = {}


def _build_nc(bv_zero=True):
    """Build + compile the single-core Bass program (shared by all 8 cores)."""
    key = ("nc", bv_zero)
    if key in _nc_cache:
        return _nc_cache[key]

    from contextlib import ExitStack

    import concourse.bacc as bacc
    import concourse.mybir as mybir
    from concourse import tile

    dt = mybir.dt
    AF = mybir.ActivationFunctionType
    OP = mybir.AluOpType

    nc = bacc.Bacc("TRN2", target_bir_lowering=False, debug=False, num_devices=8)

    # x ships as fp8 E3M4 (5 significant bits), pre-scaled by 2 on the host
    # with the weights pre-divided by 2 (exact exponent shifts) so the
    # products are unchanged. E3M4 is a full-rate matmul moving operand
    # (1.0 cycles/row, same 216ns/pass) and HALVES the x stream (8.4->4.2
    # MB), which gates the startup. Simulated end-to-end error (simulator
    # matches HW to 7 digits on the bf16 config): 1.74e-2 < 2e-2 tol.
    xt = nc.dram_tensor("xt", [D, S], dt.float8e3, kind="ExternalInput").ap()
    # wt[et, p, j*KT*P + k*P + e] = W_j.T[k*P+p, et*P+e]: one contiguous 6KB
    # row per partition per e-tile -> a single cheap DMA per e-tile.
    # j: 0=decay(Wd), 1=value(Wv), 2=gate(Wg)
    wt = nc.dram_tensor("wt", [ET, P, 3 * KT * P], dt.bfloat16, kind="ExternalInput").ap()
    # bias pre-packed host-side to [P, 4*ET]: col j*ET+e holds bias_j[e*P+p];
    # j: 0=bd, 1=bv, 2=bg, 3=-bv  (bv folded into the scan: h = h'+bv,
    # h' scans (a-1)*(xWv) with initial -bv, out = (h'+bv)*g)
    bias = nc.dram_tensor("bias", [P, 4 * ET], dt.float32, kind="ExternalInput").ap()
    # bf16 output halves the HBM write drain; the host upcasts. Rounding adds
    # <= 0.2% of |out| — far inside the tolerance.
    out = nc.dram_tensor("out", [D, S], dt.bfloat16, kind="ExternalOutput").ap()

    with tile.TileContext(nc) as tc, ExitStack() as ctx:
        xp = ctx.enter_context(tc.tile_pool(name="xp", bufs=1))
        wp = ctx.enter_context(tc.tile_pool(name="wp", bufs=1))
        bp = ctx.enter_context(tc.tile_pool(name="bp", bufs=1))
        work = ctx.enter_context(tc.tile_pool(name="work", bufs=1))
        psum = ctx.enter_context(tc.tile_pool(name="psum", bufs=1, space="PSUM"))

        # Weight slabs rotate through 3 e-tiles' worth of tags; one DMA per
        # e-tile loads all three projections (3 x 2KB rows per partition).
        wtiles = {}

        def _load_w(et, j=None):
            """Load e-tile et's weights; j=None loads all three projections
            in one DMA, j=int loads only that projection's slab (used for
            e-tile 0 so the first matmuls wait on a 256KB transfer, not
            the full 768KB)."""
            if et in wtiles:
                t = wtiles[et]
            else:
                t = wp.tile(
                    [P, 3 * KT * P], dt.bfloat16, tag=f"wt{et % 3}", name=f"w{et}"
                )
                wtiles[et] = t
            if j is None:
                nc.sync.dma_start(t[:], wt[et])
            else:
                sl = slice(j * KT * P, (j + 1) * KT * P)
                nc.sync.dma_start(t[:, sl], wt[et][:, sl])

        def wop(et, j, k):
            return wtiles[et][:, j * KT * P + k * P:(j * KT * P + k * P) + P]

        # x: column-slice tiles, each holding a column range for a k-tile
        # range (one DMA instruction covers many matmul operands; per-DMA
        # dispatch on the sync queue is ~0.6us, the startup gate). The first
        # 512 columns are split into two k-halves so the very first matmuls
        # have a small (512KB) transfer to wait on. (Starting the PE earlier
        # on finer slices was measured to LOSE ~2us: the e-tile-0 phase is
        # x-supply-bound, so an earlier start just converts free front idle
        # into mid-stream stalls with extra restart overhead.)
        XSL = [(0, 4, 0, 512), (4, 8, 0, 512), (0, 4, 512, 1024),
               (4, 8, 512, 1024), (0, 4, 1024, 2048), (4, 8, 1024, 2048),
               (0, 4, 2048, 3072), (4, 8, 2048, 3072), (0, 4, 3072, 4096),
               (4, 8, 3072, 4096)]
        xtile = [None] * len(XSL)
        xt3 = xt.rearrange("(k p) s -> p k s", p=P)

        def _load_x(i):
            k0, k1, c0, c1 = XSL[i]
            t = xp.tile(
                [P, (k1 - k0) * (c1 - c0)], dt.float8e3, tag=f"x{i}", name=f"x{i}"
            )
            nc.sync.dma_start(
                t[:].rearrange("p (k s) -> p k s", k=k1 - k0),
                xt3[:, k0:k1, c0:c1],
            )
            xtile[i] = t

        def xop(k, s):
            c = s * SCH
            for i, (k0, k1, c0, c1) in enumerate(XSL):
                if k0 <= k < k1 and c0 <= c < c1:
                    o = (k - k0) * (c1 - c0) + (c - c0)
                    return xtile[i][:, o:o + SCH]
            raise AssertionError

        # PE warm-up: TRN2 ramps the PE clock 0.65 -> 2.4 GHz over ~3us of
        # busy time. Burn the ramp on junk matmuls over a memset tile while
        # the input DMAs are still in flight (~5us; ends about when the
        # first x slices land).
        warm = bp.tile([P, 64], dt.bfloat16, name="warm")
        nc.gpsimd.memset(warm[:], 0.0)
        wps = psum.tile([64, 64], dt.float32, tag="p0", name="wps")
        for r in range(110):
            nc.tensor.matmul(wps[:], warm[:], warm[:], start=True, stop=True)

        # DMA issue order ~ first-use order; the bytes ahead of the first
        # matmul's operands gate the pipeline start. With x at fp8 the
        # stream has 2x supply slack, so the per-projection w0 slab split
        # (which LOST under bf16 x by creating supply-pacing stalls) now
        # pays: only 256KB of weights + 512KB of x gate the first matmul.
        _load_w(0, j=0)
        _load_x(0)
        _load_x(1)
        btile = bp.tile([P, 4 * ET], dt.float32)
        nc.sync.dma_start(btile[:], bias)
        # Dummy sigmoid hoists the ACT table load to kernel start.
        scratch = bp.tile([P, 1], dt.float32, name="scratch")
        nc.scalar.activation(
            scratch[:], btile[:, 0:1], AF.Sigmoid, bias=btile[:, 1:2]
        )
        _load_w(0, j=2)
        _load_x(2)
        _load_x(3)
        _load_w(0, j=1)
        for i in range(4, len(XSL)):
            _load_x(i)
        _load_w(1)
        _load_w(2)

        # Work tiles: g and om are double-buffered (parity) to break WAR
        # stalls against the previous e-tile's consumers.
        a = work.tile([P, S], dt.float32, tag="a", name="a")
        u = work.tile([P, S], dt.float32, tag="u", name="u")
        h = work.tile([P, S], dt.float32, tag="h", name="h")
        gbuf = [
            work.tile([P, S], dt.float32, tag="g0", name="g0"),
            work.tile([P, S], dt.float32, tag="g1", name="g1"),
        ]
        ombuf = [
            work.tile([P, S], dt.bfloat16, tag="om0", name="om0"),
            work.tile([P, S], dt.bfloat16, tag="om1", name="om1"),
        ]

        def mm_group_kmajor(et, j, s_order=None):
            """k-major matmul group: all NS banks accumulate over k."""
            order = list(s_order) if s_order is not None else list(range(NS))
            ps = [
                psum.tile([P, SCH], dt.float32, tag=f"p{s}", name=f"ps{s}_{et}_{j}")
                for s in range(NS)
            ]
            for k in range(KT):
                lhsT = wop(et, j, k)
                for s in order:
                    nc.tensor.matmul(
                        ps[s][:], lhsT, xop(k, s),
                        start=(k == 0), stop=(k == KT - 1),
                    )
            return ps



        def mm_bank_smajor(et, j, s, tag, name):
            """One s-bank accumulated over all k (s-major building block)."""
            t = psum.tile([P, SCH], dt.float32, tag=tag, name=name)
            for k in range(KT):
                nc.tensor.matmul(
                    t[:], wop(et, j, k), xop(k, s),
                    start=(k == 0), stop=(k == KT - 1),
                )
            return t

        def evict_sigmoid(ps, dst, et, j):
            bcol = btile[:, j * ET + et: j * ET + et + 1]
            for s in range(NS):
                sl = slice(s * SCH, (s + 1) * SCH)
                nc.scalar.activation(dst[:, sl], ps[s][:], AF.Sigmoid, bias=bcol)

        def evict_u(ps):
            for s in range(NS):
                sl = slice(s * SCH, (s + 1) * SCH)
                nc.vector.scalar_tensor_tensor(
                    u[:, sl], a[:, sl], 1.0, ps[s][:],
                    op0=OP.subtract, op1=OP.mult,
                )

        def scan_mult_store(et, g, om):
            nbv = btile[:, 3 * ET + et: 3 * ET + et + 1]   # -bv
            pbv = btile[:, 1 * ET + et: 1 * ET + et + 1]   # +bv
            # h'_t = a_t * h'_{t-1} - u'_t, h'_{-1} = -bv
            nc.vector.tensor_tensor_scan(
                h[:], a[:], u[:], nbv, op0=OP.mult, op1=OP.subtract
            )
            # out = (h' + bv) * g
            nc.vector.scalar_tensor_tensor(
                om[:], h[:], pbv, g[:], op0=OP.add, op1=OP.mult
            )
            nc.sync.dma_start(out[et * P:(et + 1) * P, :], om[:])

        # ---- e-tile 0: decay+gate interleaved s-major (PE consumes x at
        # half rate so the streaming x slices keep up), then value k-major.
        g = gbuf[0]
        bd_col = btile[:, 0 * ET + 0: 0 * ET + 1]
        bg_col = btile[:, 2 * ET + 0: 2 * ET + 1]
        for s in range(NS):
            sl = slice(s * SCH, (s + 1) * SCH)
            pa = mm_bank_smajor(0, 0, s, f"p{(2 * s) % NS}", f"pa{s}_0")
            nc.scalar.activation(a[:, sl], pa[:], AF.Sigmoid, bias=bd_col)
            pg = mm_bank_smajor(0, 2, s, f"p{(2 * s + 1) % NS}", f"pg{s}_0")
            nc.scalar.activation(g[:, sl], pg[:], AF.Sigmoid, bias=bg_col)
        ps = mm_group_kmajor(0, 1)
        evict_u(ps)
        scan_mult_store(0, g, ombuf[0])
        _load_w(3)

        # ---- e-tiles 1..ET-2: decay, value, gate (k-major).
        for et in range(1, ET - 1):
            g = gbuf[et % 2]
            ps = mm_group_kmajor(et, 0)
            evict_sigmoid(ps, a, et, 0)
            ps = mm_group_kmajor(et, 1)
            evict_u(ps)
            ps = mm_group_kmajor(et, 2)
            evict_sigmoid(ps, g, et, 2)
            scan_mult_store(et, g, ombuf[et % 2])
            if et + 3 < ET:
                _load_w(et + 3)

        # ---- last e-tile: decay (k-major), value (s-major, u-evicts trail
        # each bank, scan in two chained halves behind them), gate (s-major,
        # sigmoids trail each bank), then chunked out-multiplies + stores
        # that trail the gate evictions. Tail after the last matmul is just
        # one eviction + one small multiply + one small store.
        et = ET - 1
        g = gbuf[et % 2]
        om = ombuf[et % 2]
        bd_col = btile[:, 0 * ET + et: 0 * ET + et + 1]
        bg_col = btile[:, 2 * ET + et: 2 * ET + et + 1]
        nbv = btile[:, 3 * ET + et: 3 * ET + et + 1]
        pbv = btile[:, 1 * ET + et: 1 * ET + et + 1]

        ps = mm_group_kmajor(et, 0)
        evict_sigmoid(ps, a, et, 0)
        half = S // 2
        for s in range(NS):
            sl = slice(s * SCH, (s + 1) * SCH)
            pv = mm_bank_smajor(et, 1, s, f"p{s}", f"pv{s}_{et}")
            nc.vector.scalar_tensor_tensor(
                u[:, sl], a[:, sl], 1.0, pv[:], op0=OP.subtract, op1=OP.mult
            )
            if s == NS // 2 - 1:
                nc.vector.tensor_tensor_scan(
                    h[:, 0:half], a[:, 0:half], u[:, 0:half], nbv,
                    op0=OP.mult, op1=OP.subtract,
                )
            elif s == NS - 1:
                nc.vector.tensor_tensor_scan(
                    h[:, half:S], a[:, half:S], u[:, half:S],
                    h[:, half - 1:half], op0=OP.mult, op1=OP.subtract,
                )
        # gate s-major with the out-multiply + store interleaved per bank:
        # earlier chunks drain while later gate banks still stream, so the
        # post-last-matmul chain is only ACT(256) + mult(256) + one small
        # store. The final bank is built as two 256-col accumulation
        # groups in the same psum bank (sequential groups; the second
        # group's writes never touch the first half's bytes).
        for s in range(NS):
            sl = slice(s * SCH, (s + 1) * SCH)
            if s < NS - 1:
                pg = mm_bank_smajor(et, 2, s, f"p{s}", f"pg{s}_{et}")
                nc.scalar.activation(g[:, sl], pg[:], AF.Sigmoid, bias=bg_col)
                nc.vector.scalar_tensor_tensor(
                    om[:, sl], h[:, sl], pbv, g[:, sl],
                    op0=OP.add, op1=OP.mult,
                )
                nc.sync.dma_start(out[et * P:(et + 1) * P, sl], om[:, sl])
            else:
                t = psum.tile([P, SCH], dt.float32, tag=f"p{s}", name=f"pg{s}_{et}")
                for hf in range(2):
                    hsl = slice(s * SCH + hf * 256, s * SCH + hf * 256 + 256)
                    pslice = t[:, hf * 256:hf * 256 + 256]
                    for k in range(KT):
                        nc.tensor.matmul(
                            pslice, wop(et, 2, k),
                            xop(k, s)[:, hf * 256:hf * 256 + 256],
                            start=(k == 0), stop=(k == KT - 1),
                        )
                    nc.scalar.activation(
                        g[:, hsl], pslice, AF.Sigmoid, bias=bg_col
                    )
                    nc.vector.scalar_tensor_tensor(
                        om[:, hsl], h[:, hsl], pbv, g[:, hsl],
                        op0=OP.add, op1=OP.mult,
                    )
                    nc.sync.dma_start(out[et * P:(et + 1) * P, hsl], om[:, hsl])

    nc.compile()
    _nc_cache[key] = nc
    return nc


def _start_trace():
    """Begin an NRT/NTFF profile capture on core 0 via the axon PJRT .so.

    Dev-only (MINGRU_TRACE=1); returns None on any failure so the normal
    execution path is never affected.
    """
    try:
        import ctypes
        import tempfile

        so = "/opt/axon/libaxon_pjrt.so"
        lib = ctypes.CDLL(so)
        if not hasattr(lib, "axon_start_nrt_profile"):
            return None
        lib.axon_start_nrt_profile.argtypes = [
            ctypes.POINTER(ctypes.c_int64),
            ctypes.c_size_t,
        ]
        lib.axon_start_nrt_profile.restype = ctypes.c_int64
        lib.axon_stop_nrt_profile.argtypes = [ctypes.c_char_p]
        lib.axon_stop_nrt_profile.restype = ctypes.c_int64

        import jax

        jax.devices()
        ids = (ctypes.c_int64 * 1)(0)
        rc = lib.axon_start_nrt_profile(ids, 1)
        if rc != 0:
            print(f"trace: axon_start_nrt_profile rc={rc}")
            return None
        outdir = tempfile.mkdtemp(prefix="mingru_ntff_")
        return (lib, outdir)
    except Exception as e:
        print(f"trace: start failed: {e!r}")
        return None


def _stop_trace(tracer, nc):
    """Stop the capture, convert NTFF -> perfetto, stash BassKernelResults."""
    lib, outdir = tracer
    try:
        n = lib.axon_stop_nrt_profile(str(outdir).encode())
        print(f"trace: {n} file(s) written to {outdir}")
        if n <= 0:
            return
        import gauge.profiler
        from concourse import bass_utils
        from concourse._compat import FishPath

        profile = gauge.profiler.Profile(
            profile_path=FishPath(outdir),
            kernel_dev_mode=True,
            profile_on_exit=False,
            bass_kernel=nc.m,
            offline_processing=True,
            fname="*_body*",
            metadata={},
        )
        perf = bass_utils._process_ntff_profile(
            profile,
            outdir,
            nc,
            core_ids=list(range(B)),
            trace_cores=[0],
            stitch_traces=False,
            trace_kwargs={},
            trace_events=False,
        )
        _nc_cache["last_results"] = perf.as_bass_kernel_results([])
    except Exception as e:
        print(f"trace: postprocess failed: {e!r}")


def _run_spmd_sharded(nc, in_maps, n_cores):
    """Like bass2jax.run_bass_via_pjrt, but moves data per-shard (16MB max per
    transfer) instead of one big concatenated host<->device transfer, which
    overflows the axon tunnel at our sizes (128MB outputs)."""
    import jax
    import jax.numpy as jnp
    import concourse.mybir as mybir
    from concourse import bass2jax
    from jax.sharding import Mesh, NamedSharding, PartitionSpec
    from jax.experimental.shard_map import shard_map

    bass2jax.install_neuronx_cc_hook()

    partition_name = nc.partition_id_tensor.name if nc.partition_id_tensor else None

    in_names, out_names, out_avals = [], [], []
    for alloc in nc.m.functions[0].allocations:
        if not isinstance(alloc, mybir.MemoryLocationSet):
            continue
        name = alloc.memorylocations[0].name
        if alloc.kind == "ExternalInput":
            if name != partition_name:
                in_names.append(name)
        elif alloc.kind == "ExternalOutput":
            out_names.append(name)
            out_avals.append(
                jax.core.ShapedArray(
                    tuple(alloc.tensor_shape), mybir.dt.np(alloc.dtype)
                )
            )
    n_params = len(in_names)
    n_outs = len(out_avals)
    in_names = in_names + out_names
    if partition_name is not None:
        in_names.append(partition_name)
    donate = tuple(range(n_params, n_params + n_outs))

    def _body(*args):
        operands = list(args)
        if partition_name is not None:
            operands.append(bass2jax.partition_id_tensor())
        return tuple(
            bass2jax._bass_exec_p.bind(
                *operands,
                out_avals=tuple(out_avals),
                in_names=tuple(in_names),
                out_names=tuple(out_names),
                lowering_input_output_aliases=(),
                sim_require_finite=True,
                sim_require_nnan=True,
                nc=nc,
            )
        )

    devices = jax.devices()[:n_cores]
    mesh = Mesh(np.asarray(devices), ("core",))
    sharding = NamedSharding(mesh, PartitionSpec("core"))
    in_specs = (PartitionSpec("core"),) * (n_params + n_outs)
    out_specs = (PartitionSpec("core"),) * n_outs
    fn = jax.jit(
        shard_map(
            _body, mesh=mesh, in_specs=in_specs, out_specs=out_specs, check_rep=False
        ),
        donate_argnums=donate,
        keep_unused=True,
    )

    def put(name):
        shards = [
            jax.device_put(np.asarray(in_maps[c][name]), devices[c])
            for c in range(n_cores)
        ]
        shp = shards[0].shape
        return jax.make_array_from_single_device_arrays(
            (n_cores * shp[0], *shp[1:]), sharding, shards
        )

    args = [put(name) for name in in_names[:n_params]]
    zeros = [
        jnp.zeros((n_cores * a.shape[0], *a.shape[1:]), a.dtype, device=sharding)
        for a in out_avals
    ]

    tracer = _start_trace() if os.environ.get("MINGRU_TRACE") == "1" else None
    out_arrs = fn(*args, *zeros)
    jax.block_until_ready(out_arrs)
    if tracer is not None:
        _stop_trace(tracer, nc)

    results = [dict() for _ in range(n_cores)]
    for i, name in enumerate(out_names):
        shards = sorted(
            out_arrs[i].addressable_shards, key=lambda s: s.index[0].start or 0
        )
        assert len(shards) == n_cores
        for c in range(n_cores):
            results[c][name] = np.asarray(shards[c].data)
    return results


def kernel(x, Wg, bg, Wv, bv, Wd, bd):
    x = np.asarray(x)
    nc = _build_nc(bv_zero=not np.any(np.asarray(bv)))

    # Pack weights per (et, j): wt[et, j, p, k*P+e] = W_j.T[k*P+p, et*P+e].
    # Weights are pre-divided by 2 (exact) to compensate the x*2 pre-scale
    # applied before the fp8 E3M4 cast (uses e3m4's range/subnormal zone
    # better; x*2 stays within its +-15.5 max).
    def pack(w):
        wT = np.ascontiguousarray(np.asarray(w, np.float32).T * 0.5)  # [d, e]
        return (
            wT.reshape(KT, P, ET, P)
            .transpose(2, 1, 0, 3)
            .reshape(ET, P, KT * P)
        )

    # stack j on the last-but-one axis then flatten: [ET, P, 3*KT*P]
    wt = np.ascontiguousarray(
        np.stack([pack(Wd), pack(Wv), pack(Wg)], axis=2).reshape(
            ET, P, 3 * KT * P
        )
    ).astype(_BF16)
    bv = np.asarray(bv)
    # bias packed to [P, 4*ET]: col j*ET+e holds bias_j[e*P+p] (contiguous
    # rows -> cheap DMA descriptor generation).
    bias = np.ascontiguousarray(
        np.stack([np.asarray(bd), bv, np.asarray(bg), -bv])
        .astype(np.float32)
        .reshape(4, ET, P)
        .transpose(2, 0, 1)
        .reshape(P, 4 * ET)
    )

    in_maps = []
    for b in range(B):
        xtr = np.ascontiguousarray(
            np.asarray(x[b].T, np.float32) * 2.0
        ).astype(_E3M4)  # [D, S] fp8 e3m4, pre-scaled by 2
        in_maps.append({"xt": xtr, "wt": wt, "bias": bias})

    results = _run_spmd_sharded(nc, in_maps, n_cores=B)

    out = np.empty((B, S, D), np.float32)
    for b in range(B):
        out[b] = results[b]["out"].T.astype(np.float32)
    return out

